# revision 51
# baseline (speedup 1.0000x reference)
"""Trainium2 Bass kernel for nn_AffineAttentionNN (moe_routing).

Math (per the reference):
    dist_sq[n,c] = ||x[n]-ctrs[c]||^2_s   (s-weighted squared distance)
    a = softmax(-dist_sq, axis=c)
    out = einsum('nc,ng,cgp->np', a, x, Wv) + a @ Ov

Device decomposition (data-parallel over n across 8 cores; per core n_loc=2048):
  - Softmax offsets: the per-row term (x*x)@s is constant along c and cancels;
    we exponentiate g[c,n] = 2(x*s)@ctrs.T - ccs[c] directly.
  - All heavy tensors in bf16: the per-expert value matmul runs 1 cyc/col on
    PE, the routing-weight multiply runs 2 elem/cyc/lane on DVE (2x_1P mode),
    and the partition-broadcast DMA volume halves vs fp32.
  - Per expert c: e_c must appear on all 128 partitions to scale x. Split the
    replication between the DMA engines (partition_broadcast of PAIRS of
    adjacent e-rows -> 1 MB transfers) and the PE (one-hot matmul replicating
    eT row c into PSUM, evacuated to SBUF bf16 by the otherwise-idle ScalarE).
    This balances DMA-write vs PE vs ACT so the DVE multiply stream is the
    critical path.
      gT = matmul(lhsT=2*s*ctrs (g,c), rhs=xT) (f32r)        [c, n] PSUM
      eT = Exp(gT + bias=-ccs[c])  on ScalarE -> bf16        [c, n]
      Z  = matmul(lhsT=ones, rhs=eT) -> reciprocal           [1, n]
      per expert c: er = broadcast(e[c,:]) via DMA or PE+ACT [g, n] bf16
                    z  = xT_bf * er   (VectorE, bf16 2x)     [g, n]
                    outT[p, js] += matmul(Wv[c] (g,p), z)    PSUM acc
      outT += matmul(lhsT=Ov (c,p), rhs=eT)                  (Ov term)
      out  = outT * recipZ_rep  (normalize), DMA out, host transposes.
"""

import os
import numpy as np
from contextlib import ExitStack

import concourse.bass as bass
import concourse.tile as tile
from concourse import mybir

N, D, C, P = 16384, 128, 128, 128
N_CORES = 8
N_LOC = N // N_CORES          # 2048
CHUNK = 512                   # PSUM bank width (fp32)
NCH = N_LOC // CHUNK          # 4

F32 = mybir.dt.float32
F32R = mybir.dt.float32r
BF16 = mybir.dt.bfloat16

# Number of expert PAIRS whose e-rows are replicated on the PE (one-hot
# matmul into PSUM + ScalarE evacuation) instead of by broadcast DMA.
N_PAIRS = C // 2
PE_PAIRS = int(os.environ.get("KERNEL_PE_PAIRS", "24"))
# How many experts ahead the PE replication runs of the consuming multiply.
LOOKAHEAD = int(os.environ.get("KERNEL_LOOKAHEAD", "14"))
# Leading pairs forced onto the PE-replication path: their supply comes from
# eT in SBUF, hiding the e->DRAM->broadcast latency at kernel start.
FRONT_PE = int(os.environ.get("KERNEL_FRONT_PE", "4"))
# Trailing pairs on the PE path: by the end the consumer has caught up with
# the broadcast stream, and PE-supplied pairs come straight from SBUF.
TAIL_PE = int(os.environ.get("KERNEL_TAIL_PE", "3"))
# Pairs whose routing-weight multiply runs on the (otherwise idle) GPSIMD
# engine instead of the DVE. tensor_tensor on DVE is a single-port op, so the
# two engines genuinely overlap.
GP_PAIRS = int(os.environ.get("KERNEL_GP_PAIRS", "0"))
# Evacuate the FRONT pairs' PE-replicated e-rows on the DVE (idle during the
# ramp; 660 ns/half vs ScalarE's ~1080) to shorten the startup supply chain.
FRONT_DVE = os.environ.get("KERNEL_FRONT_DVE", "1") == "1"


def _pair_is_gp(p):
    if p < FRONT_PE:
        return False
    return (p - FRONT_PE) % max(1, (N_PAIRS - FRONT_PE) // max(1, GP_PAIRS)) == 2 \
        and sum(1 for q in range(FRONT_PE, p)
                if (q - FRONT_PE) % max(1, (N_PAIRS - FRONT_PE) // max(1, GP_PAIRS)) == 2) < GP_PAIRS


def _pair_is_pe(p):
    """FRONT_PE leading + TAIL_PE trailing pairs, Bresenham spread between."""
    if p < FRONT_PE or p >= N_PAIRS - TAIL_PE:
        return True
    rest = PE_PAIRS - FRONT_PE - TAIL_PE
    span = N_PAIRS - FRONT_PE - TAIL_PE
    q = p - FRONT_PE
    return ((q + 1) * rest) // span - (q * rest) // span == 1


PE_EXPERTS = sorted(c for p in range(N_PAIRS) if _pair_is_pe(p) for c in (2 * p, 2 * p + 1))
PE_SLOT = {c: i for i, c in enumerate(PE_EXPERTS)}


def _dedup_ldweights(nc):
    """Each matmul is emitted as an InstLdweights + non-self-loading
    InstMatmult pair; a run of matmuls sharing the same stationary operand
    re-loads it before every matmul, which blocks fill/drain overlap between
    them (379 ns/MM instead of ~216). Delete the redundant loads. The
    schedule is final here, so block order IS the PE execution order; the
    deleted loads carry no sync_info and no one references them. bf16-only
    out of caution (f32r has a known walrus quirk around non-self-loading)."""
    n = 0
    for f in nc.m.functions:
        for blk in f.blocks:
            last_sig = None
            keep = []
            for inst in blk.instructions:
                if str(inst.engine) != "EngineType.PE":
                    keep.append(inst)
                    continue
                if isinstance(inst, mybir.InstLdweights):
                    w = inst.ins[0]
                    si = inst.sync_info
                    clean = si is None or (not si.on_wait and not si.on_update)
                    if (w.dtype == mybir.dt.bfloat16 and clean
                            and inst.perf_mode is None
                            and inst.is_transpose is None):
                        sig = (str(w.ap), w.offset, str(w.memref))
                        if sig == last_sig:
                            n += 1
                            continue  # drop the redundant load
                        last_sig = sig
                    else:
                        last_sig = None
                elif isinstance(inst, mybir.InstMatmult):
                    if inst.ldweights is not False or inst.is_transpose:
                        last_sig = None  # self-loading matmul replaces weights
                elif isinstance(inst, (mybir.InstEventSemaphore, mybir.InstDrain)):
                    pass  # no effect on the loaded weights
                else:
                    last_sig = None
                keep.append(inst)
            blk.instructions = keep
    return n


def _legalize_waits(nc, max_waits=1):
    """This walrus build accepts at most one sync-wait per instruction; Tile
    emits several. Hoist the excess onto standalone single-wait
    InstEventSemaphore ops just before the owner on the same engine stream."""
    import bass_rust

    n = 0
    for f in nc.m.functions:
        for blk in f.blocks:
            out = []
            for inst in blk.instructions:
                si = getattr(inst, "sync_info", None)
                waits = list(si.on_wait) if si is not None else []
                if len(waits) > max_waits:
                    extra, keep = waits[:-max_waits], waits[-max_waits:]
                    for w in extra:
                        n += 1
                        ev = mybir.InstEventSemaphore(
                            name=f"legal_wait_{n}_{inst.name}", ins=[], outs=[]
                        )
                        ev.engine = inst.engine
                        ev.sync_info = bass_rust.SyncInfo(on_wait=[w], on_update=[])
                        out.append(ev)
                    inst.sync_info = bass_rust.SyncInfo(
                        on_wait=keep, on_update=list(si.on_update)
                    )
                out.append(inst)
            blk.instructions = out
    return n


def _emit_kernel(tc, aps):
    nc = tc.nc
    xT, xTb, wvT, selT, c2sT, nccs, ov, outT = (
        aps["xT"], aps["xTb"], aps["wvT"], aps["selT"], aps["c2sT"],
        aps["nccs"], aps["ov"], aps["outT"],
    )

    with ExitStack() as ctx:
        const = ctx.enter_context(tc.tile_pool(name="const", bufs=1))
        dram = ctx.enter_context(tc.tile_pool(name="dram", bufs=1, space="DRAM"))
        erep_p = ctx.enter_context(tc.tile_pool(name="erep", bufs=7))
        erpe_p = ctx.enter_context(tc.tile_pool(name="erpe", bufs=4))
        z_p = ctx.enter_context(tc.tile_pool(name="zt", bufs=4))
        out_p = ctx.enter_context(tc.tile_pool(name="outs", bufs=1))

        # ---- constants / inputs into SBUF ----
        # Order matters: the prologue (distance matmul -> exp) gates the whole
        # pipeline, so its inputs load first on the sync ring; the bulky Wv
        # rides the SWDGE (gpsimd) ring concurrently and is only needed once
        # the first z tile exists.
        c2s_s = const.tile([D, C], F32R, tag="c2s")
        nc.sync.dma_start(c2s_s[:], c2sT[:, :])
        nccs_s = const.tile([C, 1], F32, tag="nccs")
        nc.sync.dma_start(nccs_s[:], nccs[:, :])
        xT_s = const.tile([D, N_LOC], F32R, tag="xT")
        for j in range(NCH):
            js = slice(j * CHUNK, (j + 1) * CHUNK)
            nc.sync.dma_start(xT_s[:, js], xT[:, js])
        id_s = const.tile([C, C], BF16, tag="id")
        nc.sync.dma_start(id_s[:], selT[:, :])
        xTb_s = const.tile([D, N_LOC], BF16, tag="xTb")
        nc.sync.dma_start(xTb_s[:], xTb[:, :])
        ov_s = const.tile([C, P], BF16, tag="ov")
        nc.sync.dma_start(ov_s[:], ov[:, :])
        WVCH = C * P // 8
        wv_s = const.tile([D, C * P], BF16, tag="wv")
        for k in range(8):
            nc.gpsimd.dma_start(
                wv_s[:, k * WVCH:(k + 1) * WVCH], wvT[:, k * WVCH:(k + 1) * WVCH])
        ones_s = const.tile([C, 1], BF16, tag="ones")
        nc.vector.memset(ones_s[:], 1.0)
        # prewarm the exp table so ACT_TABLE_LOAD isn't serialized into the
        # first real activation's dependency chain
        warm_s = const.tile([C, 1], F32, tag="warm")
        nc.scalar.activation(
            warm_s[:], ones_s[:], mybir.ActivationFunctionType.Exp)
        eT_s = const.tile([C, N_LOC], BF16, tag="eT")
        rz_s = const.tile([1, N_LOC], F32, tag="rz")
        zc_s = const.tile([P, N_LOC // P], F32, tag="zc")
        zcr_s = const.tile([P, N_LOC // P], F32, tag="zcr")
        rzrep_s = const.tile([P, N_LOC], F32, tag="rzrep")

        e_dram = dram.tile([N_PAIRS, 2 * N_LOC], BF16, tag="e_dram")
        rz_dram = dram.tile([1, N_LOC], F32, tag="rz_dram")
        rzr_dram = dram.tile([1, N_LOC], F32, tag="rzr_dram")

        # ---- prologue: distances -> unnormalized softmax weights eT [c, n] ----
        with ExitStack() as dctx:
            psum_d = dctx.enter_context(
                tc.tile_pool(name="psum_d", bufs=2, space="PSUM"))
            psum_z = dctx.enter_context(
                tc.tile_pool(name="psum_z", bufs=1, space="PSUM"))
            for j in range(NCH):
                js = slice(j * CHUNK, (j + 1) * CHUNK)
                pd = psum_d.tile([C, CHUNK], F32, tag="pd")
                nc.tensor.matmul(pd[:], c2s_s[:], xT_s[:, js], start=True, stop=True)
                nc.scalar.activation(
                    eT_s[:, js], pd[:], mybir.ActivationFunctionType.Exp,
                    bias=nccs_s[:, 0:1], scale=1.0,
                )
                pz = psum_z.tile([1, CHUNK], F32, tag="pz")
                nc.tensor.matmul(pz[:], ones_s[:], eT_s[:, js], start=True, stop=True)
                nc.scalar.copy(rz_s[0:1, js], pz[0:1, :])

        # e -> DRAM (paired-row layout) for the partition-broadcast DMAs;
        # 1/Z on all 128 lanes via a strided-DMA transpose roundtrip. The rz
        # chain rides the SWDGE ring: its writes wait on compute, and on the
        # sync FIFO they would block every pair-broadcast queued behind them.
        e_flat = e_dram[:, :].rearrange("a (b n) -> (a b) n", b=2)
        nc.sync.dma_start(e_flat, eT_s[:])
        nc.gpsimd.dma_start(rz_dram[:, :], rz_s[:])
        nc.gpsimd.dma_start(
            zc_s[:], rz_dram[0:1, :].rearrange("o (f p) -> (o p) f", p=P))
        nc.vector.reciprocal(zcr_s[:], zc_s[:])
        nc.gpsimd.dma_start(
            rzr_dram[0:1, :].rearrange("o (f p) -> (o p) f", p=P), zcr_s[:])
        nc.gpsimd.dma_start(rzrep_s[:], rzr_dram[0:1, :].partition_broadcast(P))

        # ---- main expert loop, accumulate outT in PSUM ----
        psum_o = ctx.enter_context(tc.tile_pool(name="psum_o", bufs=1, space="PSUM"))
        psum_r = ctx.enter_context(tc.tile_pool(name="psum_r", bufs=2, space="PSUM"))
        po = psum_o.tile([P, N_LOC], F32, tag="po")

        er_pair = {}   # pair -> er2 tile [D, 2*N_LOC]
        REP_HALF = N_LOC // 2
        # x repeated twice along a stride-0 dim, for one multiply per pair
        x_rep2 = xTb_s[:].unsqueeze(1).broadcast_to([D, 2, N_LOC])

        def emit_replicate(c):
            """PE one-hot matmul: replicate eT row c to all partitions, in two
            [D, N_LOC/2] PSUM tiles, each evacuated to SBUF bf16. Front pairs
            evacuate on the (ramp-idle) DVE for a faster startup cadence;
            tail pairs are produced early into their own pool; the rest
            evacuate on ScalarE. The two experts of a pair share one
            [D, 2*N_LOC] SBUF tile."""
            p, half = divmod(c, 2)
            if half == 0:
                erpe = erpe_p.tile([D, 2 * N_LOC], BF16, tag="erpe")
                er_pair[p] = erpe
            er = er_pair[p]
            # one-hot weights: identity column c repeated via a stride-0 dim
            sel_c = id_s[:, c:c + 1].broadcast_to([C, P])
            for h in range(2):
                dst = slice(half * N_LOC + h * REP_HALF,
                            half * N_LOC + (h + 1) * REP_HALF)
                pr = psum_r.tile([D, REP_HALF], F32, tag="pr")
                for q in range(REP_HALF // CHUNK):
                    qs = slice(q * CHUNK, (q + 1) * CHUNK)
                    src = slice(h * REP_HALF + q * CHUNK,
                                h * REP_HALF + (q + 1) * CHUNK)
                    nc.tensor.matmul(
                        pr[:, qs], sel_c, eT_s[:, src], start=True, stop=True)
                if FRONT_DVE and p < FRONT_PE:
                    nc.vector.tensor_copy(er[:, dst], pr[:])
                else:
                    nc.scalar.copy(er[:, dst], pr[:])

        def emit_broadcast_pair(p):
            """One 1 MB DMA: broadcast e-rows (2p, 2p+1) to 128 partitions.
            HWDGE only — SWDGE would serialize behind GPSIMD tensor ops."""
            er2 = erep_p.tile([D, 2 * N_LOC], BF16, tag="er2")
            nc.sync.dma_start(er2[:], e_dram[p:p + 1, :].partition_broadcast(D))
            er_pair[p] = er2

        # Producer schedule: front pairs, then the tail pairs (produced early
        # into their own pool so the end of the broadcast stream can't starve
        # the consumer), then everything else in pair order.
        sched = []
        for p in range(N_PAIRS):
            if _pair_is_pe(p):
                sched.append(("rep", 2 * p, True))
                sched.append(("rep", 2 * p + 1, True))
            else:
                sched.append(("bc", p, True))
        prod_i = 0

        def pump(c_needed):
            nonlocal prod_i
            covered = -1
            while prod_i < len(sched) and covered < min(c_needed + LOOKAHEAD, C - 1):
                kind, v, counts = sched[prod_i]
                if kind == "bc":
                    emit_broadcast_pair(v)
                    covered = 2 * v + 1
                else:
                    emit_replicate(v)
                    if counts:
                        covered = v if v % 2 == 1 else v - 1
                prod_i += 1

        for p in range(N_PAIRS):
            pump(2 * p + 1)
            er2 = er_pair.pop(p)
            z2 = z_p.tile([D, 2 * N_LOC], BF16, tag="z2")
            if p < FRONT_PE:
                # startup: one multiply per expert so the first matmuls don't
                # wait for the whole pair's replication
                nc.vector.tensor_mul(z2[:, 0:N_LOC], xTb_s[:], er2[:, 0:N_LOC])
                nc.vector.tensor_mul(z2[:, N_LOC:], xTb_s[:], er2[:, N_LOC:])
            elif _pair_is_gp(p):
                nc.gpsimd.tensor_mul(z2[:], x_rep2, er2[:])
            else:
                nc.vector.tensor_mul(z2[:], x_rep2, er2[:])
            for half in range(2):
                c = 2 * p + half
                wv_c = wv_s[:, c * P:(c + 1) * P]
                for j in range(NCH):
                    js = slice(half * N_LOC + j * CHUNK,
                               half * N_LOC + (j + 1) * CHUNK)
                    ps = slice(j * CHUNK, (j + 1) * CHUNK)
                    nc.tensor.matmul(
                        po[:, ps], wv_c, z2[:, js],
                        start=(c == 0), stop=False, skip_group_check=True,
                    )

        # ---- Ov term closes each chunk's accumulation group; normalize and
        # store per chunk so the tail pipelines across PE/DVE/DMA ----
        out_s = out_p.tile([P, N_LOC], F32, tag="out")
        for j in range(NCH):
            js = slice(j * CHUNK, (j + 1) * CHUNK)
            nc.tensor.matmul(
                po[:, js], ov_s[:], eT_s[:, js],
                start=False, stop=True, skip_group_check=True,
            )
            nc.vector.tensor_mul(out_s[:, js], po[:, js], rzrep_s[:, js])
            nc.sync.dma_start(outT[:, js], out_s[:, js])


def build_nc():
    nc = bass.Bass(target_bir_lowering=False, trn_type="TRN2")
    aps = {
        "xT": nc.dram_tensor("xT", [D, N_LOC], F32R, kind="ExternalInput").ap(),
        "xTb": nc.dram_tensor("xTb", [D, N_LOC], BF16, kind="ExternalInput").ap(),
        "wvT": nc.dram_tensor("wvT", [D, C * P], BF16, kind="ExternalInput").ap(),
        "selT": nc.dram_tensor("selT", [C, C], BF16, kind="ExternalInput").ap(),
        "c2sT": nc.dram_tensor("c2sT", [D, C], F32R, kind="ExternalInput").ap(),
        "nccs": nc.dram_tensor("nccs", [C, 1], F32, kind="ExternalInput").ap(),
        "ov": nc.dram_tensor("ov", [C, P], BF16, kind="ExternalInput").ap(),
        "outT": nc.dram_tensor("outT", [P, N_LOC], F32, kind="ExternalOutput").ap(),
    }
    with tile.TileContext(nc) as tc:
        _emit_kernel(tc, aps)
    _dedup_ldweights(nc)
    _legalize_waits(nc)
    return nc


_CACHE = {}


def _get_nc():
    key = (PE_PAIRS, LOOKAHEAD)
    if key not in _CACHE:
        _CACHE[key] = build_nc()
    return _CACHE[key]


def kernel(x, ctrs, Wv, Ov, s, _spmd_kwargs=None):
    import ml_dtypes
    from concourse.bass_utils import run_bass_kernel_spmd

    bf16 = ml_dtypes.bfloat16
    x = np.ascontiguousarray(x, dtype=np.float32)
    ctrs = np.asarray(ctrs, dtype=np.float32)
    Wv = np.asarray(Wv, dtype=np.float32)
    Ov = np.ascontiguousarray(Ov, dtype=np.float32)
    s = np.asarray(s, dtype=np.float32)

    # host-side prep of the small routing constants
    c2sT = np.ascontiguousarray((2.0 * ctrs * s[None, :]).T)        # [g, c]
    nccs = np.ascontiguousarray(-((ctrs * ctrs) @ s)[:, None])      # [c, 1]
    wvT = np.ascontiguousarray(
        Wv.transpose(1, 0, 2).reshape(D, C * P).astype(bf16))       # [g, c*p]
    ov_b = np.ascontiguousarray(Ov.astype(bf16))
    selT = np.eye(C, dtype=np.float32).astype(bf16)

    in_maps = []
    for i in range(N_CORES):
        xi = x[i * N_LOC:(i + 1) * N_LOC]
        xiT = np.ascontiguousarray(xi.T)
        m = {
            "xT": xiT,
            "xTb": np.ascontiguousarray(xiT.astype(bf16)),
            "wvT": wvT,
            "selT": selT,
            "c2sT": c2sT,
            "nccs": nccs,
            "ov": ov_b,
        }
        in_maps.append(m)

    nc = _get_nc()
    for attempt in range(3):
        res = run_bass_kernel_spmd(
            nc, in_maps, core_ids=list(range(N_CORES)), **(_spmd_kwargs or {})
        )
        out = np.empty((N, P), dtype=np.float32)
        for i in range(N_CORES):
            out[i * N_LOC:(i + 1) * N_LOC] = res.results[i]["outT"].T
        kernel.last_result = res
        if np.isfinite(out).all():
            break
    return out


# revision 52
# speedup vs baseline: 1.2557x; 1.2557x over previous
"""Trainium2 Bass kernel for nn_AffineAttentionNN (moe_routing).

Math (per the reference):
    dist_sq[n,c] = ||x[n]-ctrs[c]||^2_s   (s-weighted squared distance)
    a = softmax(-dist_sq, axis=c)
    out = einsum('nc,ng,cgp->np', a, x, Wv) + a @ Ov

Device decomposition (data-parallel over n across 8 cores; per core n_loc=2048):
  - Softmax offsets: the per-row term (x*x)@s is constant along c and cancels;
    we exponentiate g[c,n] = 2(x*s)@ctrs.T - ccs[c] directly.
  - All heavy tensors in bf16: the per-expert value matmul runs 1 cyc/col on
    PE, the routing-weight multiply runs 2 elem/cyc/lane on DVE (2x_1P mode),
    and the partition-broadcast DMA volume halves vs fp32.
  - Per expert c: e_c must appear on all 128 partitions to scale x. Split the
    replication between the DMA engines (partition_broadcast of PAIRS of
    adjacent e-rows -> 1 MB transfers) and the PE (one-hot matmul replicating
    eT row c into PSUM, evacuated to SBUF bf16 by the otherwise-idle ScalarE).
    This balances DMA-write vs PE vs ACT so the DVE multiply stream is the
    critical path.
      gT = matmul(lhsT=2*s*ctrs (g,c), rhs=xT) (f32r)        [c, n] PSUM
      eT = Exp(gT + bias=-ccs[c])  on ScalarE -> bf16        [c, n]
      Z  = matmul(lhsT=ones, rhs=eT) -> reciprocal           [1, n]
      per expert c: er = broadcast(e[c,:]) via DMA or PE+ACT [g, n] bf16
                    z  = xT_bf * er   (VectorE, bf16 2x)     [g, n]
                    outT[p, js] += matmul(Wv[c] (g,p), z)    PSUM acc
      outT += matmul(lhsT=Ov (c,p), rhs=eT)                  (Ov term)
      out  = outT * recipZ_rep  (normalize), DMA out, host transposes.
"""

import os
import numpy as np
from contextlib import ExitStack

import concourse.bass as bass
import concourse.tile as tile
from concourse import mybir

N, D, C, P = 16384, 128, 128, 128
N_CORES = 8
N_LOC = N // N_CORES          # 2048
CHUNK = 512                   # PSUM bank width (fp32)
NCH = N_LOC // CHUNK          # 4

F32 = mybir.dt.float32
F32R = mybir.dt.float32r
BF16 = mybir.dt.bfloat16

# Number of expert PAIRS whose e-rows are replicated on the PE (one-hot
# matmul into PSUM + ScalarE evacuation) instead of by broadcast DMA.
N_PAIRS = C // 2
PE_PAIRS = int(os.environ.get("KERNEL_PE_PAIRS", "24"))
# How many experts ahead the PE replication runs of the consuming multiply.
LOOKAHEAD = int(os.environ.get("KERNEL_LOOKAHEAD", "14"))
# Leading pairs forced onto the PE-replication path: their supply comes from
# eT in SBUF, hiding the e->DRAM->broadcast latency at kernel start.
FRONT_PE = int(os.environ.get("KERNEL_FRONT_PE", "4"))
# Trailing pairs on the PE path: by the end the consumer has caught up with
# the broadcast stream, and PE-supplied pairs come straight from SBUF.
TAIL_PE = int(os.environ.get("KERNEL_TAIL_PE", "3"))
# Pairs whose routing-weight multiply runs on the (otherwise idle) GPSIMD
# engine instead of the DVE. tensor_tensor on DVE is a single-port op, so the
# two engines genuinely overlap.
GP_PAIRS = int(os.environ.get("KERNEL_GP_PAIRS", "0"))
# Evacuate the FRONT pairs' PE-replicated e-rows on the DVE (idle during the
# ramp; 660 ns/half vs ScalarE's ~1080) to shorten the startup supply chain.
FRONT_DVE = os.environ.get("KERNEL_FRONT_DVE", "0") == "1"


def _pair_is_gp(p):
    if p < FRONT_PE:
        return False
    return (p - FRONT_PE) % max(1, (N_PAIRS - FRONT_PE) // max(1, GP_PAIRS)) == 2 \
        and sum(1 for q in range(FRONT_PE, p)
                if (q - FRONT_PE) % max(1, (N_PAIRS - FRONT_PE) // max(1, GP_PAIRS)) == 2) < GP_PAIRS


def _pair_is_pe(p):
    """FRONT_PE leading + TAIL_PE trailing pairs, Bresenham spread between."""
    if p < FRONT_PE or p >= N_PAIRS - TAIL_PE:
        return True
    rest = PE_PAIRS - FRONT_PE - TAIL_PE
    span = N_PAIRS - FRONT_PE - TAIL_PE
    q = p - FRONT_PE
    return ((q + 1) * rest) // span - (q * rest) // span == 1


PE_EXPERTS = sorted(c for p in range(N_PAIRS) if _pair_is_pe(p) for c in (2 * p, 2 * p + 1))
PE_SLOT = {c: i for i, c in enumerate(PE_EXPERTS)}


def _dedup_ldweights(nc):
    """Each matmul is emitted as an InstLdweights + non-self-loading
    InstMatmult pair; a run of matmuls sharing the same stationary operand
    re-loads it before every matmul, which blocks fill/drain overlap between
    them (379 ns/MM instead of ~216). Delete the redundant loads. The
    schedule is final here, so block order IS the PE execution order; the
    deleted loads carry no sync_info and no one references them. bf16-only
    out of caution (f32r has a known walrus quirk around non-self-loading)."""
    n = 0
    for f in nc.m.functions:
        for blk in f.blocks:
            last_sig = None
            keep = []
            for inst in blk.instructions:
                if str(inst.engine) != "EngineType.PE":
                    keep.append(inst)
                    continue
                if isinstance(inst, mybir.InstLdweights):
                    w = inst.ins[0]
                    si = inst.sync_info
                    clean = si is None or (not si.on_wait and not si.on_update)
                    if (w.dtype == mybir.dt.bfloat16 and clean
                            and inst.perf_mode is None
                            and inst.is_transpose is None):
                        sig = (str(w.ap), w.offset, str(w.memref))
                        if sig == last_sig:
                            n += 1
                            continue  # drop the redundant load
                        last_sig = sig
                    else:
                        last_sig = None
                elif isinstance(inst, mybir.InstMatmult):
                    if inst.ldweights is not False or inst.is_transpose:
                        last_sig = None  # self-loading matmul replaces weights
                elif isinstance(inst, (mybir.InstEventSemaphore, mybir.InstDrain)):
                    pass  # no effect on the loaded weights
                else:
                    last_sig = None
                keep.append(inst)
            blk.instructions = keep
    return n


def _legalize_waits(nc, max_waits=1):
    """This walrus build accepts at most one sync-wait per instruction; Tile
    emits several. Hoist the excess onto standalone single-wait
    InstEventSemaphore ops just before the owner on the same engine stream."""
    import bass_rust

    n = 0
    for f in nc.m.functions:
        for blk in f.blocks:
            out = []
            for inst in blk.instructions:
                si = getattr(inst, "sync_info", None)
                waits = list(si.on_wait) if si is not None else []
                if len(waits) > max_waits:
                    extra, keep = waits[:-max_waits], waits[-max_waits:]
                    for w in extra:
                        n += 1
                        ev = mybir.InstEventSemaphore(
                            name=f"legal_wait_{n}_{inst.name}", ins=[], outs=[]
                        )
                        ev.engine = inst.engine
                        ev.sync_info = bass_rust.SyncInfo(on_wait=[w], on_update=[])
                        out.append(ev)
                    inst.sync_info = bass_rust.SyncInfo(
                        on_wait=keep, on_update=list(si.on_update)
                    )
                out.append(inst)
            blk.instructions = out
    return n


def _emit_kernel(tc, aps):
    nc = tc.nc
    xT, xTb, wvT, selT, c2sT, nccs, ov, outT = (
        aps["xT"], aps["xTb"], aps["wvT"], aps["selT"], aps["c2sT"],
        aps["nccs"], aps["ov"], aps["outT"],
    )

    with ExitStack() as ctx:
        const = ctx.enter_context(tc.tile_pool(name="const", bufs=1))
        dram = ctx.enter_context(tc.tile_pool(name="dram", bufs=1, space="DRAM"))
        erep_p = ctx.enter_context(tc.tile_pool(name="erep", bufs=7))
        erpe_p = ctx.enter_context(tc.tile_pool(name="erpe", bufs=4))
        z_p = ctx.enter_context(tc.tile_pool(name="zt", bufs=4))
        out_p = ctx.enter_context(tc.tile_pool(name="outs", bufs=1))

        # ---- constants / inputs into SBUF ----
        # Order matters: the prologue (distance matmul -> exp) gates the whole
        # pipeline, so its inputs load first on the sync ring; the bulky Wv
        # rides the SWDGE (gpsimd) ring concurrently and is only needed once
        # the first z tile exists.
        c2s_s = const.tile([D, C], F32R, tag="c2s")
        nc.sync.dma_start(c2s_s[:], c2sT[:, :])
        nccs_s = const.tile([C, 1], F32, tag="nccs")
        nc.sync.dma_start(nccs_s[:], nccs[:, :])
        xT_s = const.tile([D, N_LOC], F32R, tag="xT")
        for j in range(NCH):
            js = slice(j * CHUNK, (j + 1) * CHUNK)
            nc.sync.dma_start(xT_s[:, js], xT[:, js])
        id_s = const.tile([C, C], BF16, tag="id")
        nc.sync.dma_start(id_s[:], selT[:, :])
        xTb_s = const.tile([D, N_LOC], BF16, tag="xTb")
        nc.sync.dma_start(xTb_s[:], xTb[:, :])
        ov_s = const.tile([C, P], BF16, tag="ov")
        nc.sync.dma_start(ov_s[:], ov[:, :])
        WVCH = C * P // 8
        wv_s = const.tile([D, C * P], BF16, tag="wv")
        for k in range(8):
            nc.gpsimd.dma_start(
                wv_s[:, k * WVCH:(k + 1) * WVCH], wvT[:, k * WVCH:(k + 1) * WVCH])
        ones_s = const.tile([C, 1], BF16, tag="ones")
        nc.vector.memset(ones_s[:], 1.0)
        # prewarm the exp table so ACT_TABLE_LOAD isn't serialized into the
        # first real activation's dependency chain
        warm_s = const.tile([C, 1], F32, tag="warm")
        nc.scalar.activation(
            warm_s[:], ones_s[:], mybir.ActivationFunctionType.Exp)
        eT_s = const.tile([C, N_LOC], BF16, tag="eT")
        rz_s = const.tile([1, N_LOC], F32, tag="rz")
        zc_s = const.tile([P, N_LOC // P], F32, tag="zc")
        zcr_s = const.tile([P, N_LOC // P], F32, tag="zcr")
        rzrep_s = const.tile([P, N_LOC], F32, tag="rzrep")

        e_dram = dram.tile([N_PAIRS, 2 * N_LOC], BF16, tag="e_dram")
        rz_dram = dram.tile([1, N_LOC], F32, tag="rz_dram")
        rzr_dram = dram.tile([1, N_LOC], F32, tag="rzr_dram")

        # ---- prologue: distances -> unnormalized softmax weights eT [c, n] ----
        with ExitStack() as dctx:
            psum_d = dctx.enter_context(
                tc.tile_pool(name="psum_d", bufs=2, space="PSUM"))
            psum_z = dctx.enter_context(
                tc.tile_pool(name="psum_z", bufs=1, space="PSUM"))
            for j in range(NCH):
                js = slice(j * CHUNK, (j + 1) * CHUNK)
                pd = psum_d.tile([C, CHUNK], F32, tag="pd")
                nc.tensor.matmul(pd[:], c2s_s[:], xT_s[:, js], start=True, stop=True)
                nc.scalar.activation(
                    eT_s[:, js], pd[:], mybir.ActivationFunctionType.Exp,
                    bias=nccs_s[:, 0:1], scale=1.0,
                )
                pz = psum_z.tile([1, CHUNK], F32, tag="pz")
                nc.tensor.matmul(pz[:], ones_s[:], eT_s[:, js], start=True, stop=True)
                nc.scalar.copy(rz_s[0:1, js], pz[0:1, :])

        # e -> DRAM (paired-row layout) for the partition-broadcast DMAs;
        # 1/Z on all 128 lanes via a strided-DMA transpose roundtrip. The rz
        # chain rides the SWDGE ring: its writes wait on compute, and on the
        # sync FIFO they would block every pair-broadcast queued behind them.
        e_flat = e_dram[:, :].rearrange("a (b n) -> (a b) n", b=2)
        nc.sync.dma_start(e_flat, eT_s[:])
        nc.gpsimd.dma_start(rz_dram[:, :], rz_s[:])
        nc.gpsimd.dma_start(
            zc_s[:], rz_dram[0:1, :].rearrange("o (f p) -> (o p) f", p=P))
        nc.vector.reciprocal(zcr_s[:], zc_s[:])
        nc.gpsimd.dma_start(
            rzr_dram[0:1, :].rearrange("o (f p) -> (o p) f", p=P), zcr_s[:])
        nc.gpsimd.dma_start(rzrep_s[:], rzr_dram[0:1, :].partition_broadcast(P))

        # ---- main expert loop, accumulate outT in PSUM ----
        psum_o = ctx.enter_context(tc.tile_pool(name="psum_o", bufs=1, space="PSUM"))
        psum_r = ctx.enter_context(tc.tile_pool(name="psum_r", bufs=2, space="PSUM"))
        po = psum_o.tile([P, N_LOC], F32, tag="po")

        er_pair = {}   # pair -> er2 tile [D, 2*N_LOC]
        REP_HALF = N_LOC // 2
        # x repeated twice along a stride-0 dim, for one multiply per pair
        x_rep2 = xTb_s[:].unsqueeze(1).broadcast_to([D, 2, N_LOC])

        def emit_replicate(c):
            """PE one-hot matmul: replicate eT row c to all partitions, in two
            [D, N_LOC/2] PSUM tiles, each evacuated to SBUF bf16. Front pairs
            evacuate on the (ramp-idle) DVE for a faster startup cadence;
            tail pairs are produced early into their own pool; the rest
            evacuate on ScalarE. The two experts of a pair share one
            [D, 2*N_LOC] SBUF tile."""
            p, half = divmod(c, 2)
            if half == 0:
                erpe = erpe_p.tile([D, 2 * N_LOC], BF16, tag="erpe")
                er_pair[p] = erpe
            er = er_pair[p]
            # one-hot weights: identity column c repeated via a stride-0 dim
            sel_c = id_s[:, c:c + 1].broadcast_to([C, P])
            for h in range(2):
                dst = slice(half * N_LOC + h * REP_HALF,
                            half * N_LOC + (h + 1) * REP_HALF)
                pr = psum_r.tile([D, REP_HALF], F32, tag="pr")
                for q in range(REP_HALF // CHUNK):
                    qs = slice(q * CHUNK, (q + 1) * CHUNK)
                    src = slice(h * REP_HALF + q * CHUNK,
                                h * REP_HALF + (q + 1) * CHUNK)
                    nc.tensor.matmul(
                        pr[:, qs], sel_c, eT_s[:, src], start=True, stop=True)
                if FRONT_DVE and p < FRONT_PE:
                    nc.vector.tensor_copy(er[:, dst], pr[:])
                else:
                    nc.scalar.copy(er[:, dst], pr[:])

        def emit_broadcast_pair(p):
            """One 1 MB DMA: broadcast e-rows (2p, 2p+1) to 128 partitions.
            HWDGE only — SWDGE would serialize behind GPSIMD tensor ops."""
            er2 = erep_p.tile([D, 2 * N_LOC], BF16, tag="er2")
            nc.sync.dma_start(er2[:], e_dram[p:p + 1, :].partition_broadcast(D))
            er_pair[p] = er2

        # Producer schedule: front pairs, then the tail pairs (produced early
        # into their own pool so the end of the broadcast stream can't starve
        # the consumer), then everything else in pair order.
        sched = []
        for p in range(N_PAIRS):
            if _pair_is_pe(p):
                sched.append(("rep", 2 * p, True))
                sched.append(("rep", 2 * p + 1, True))
            else:
                sched.append(("bc", p, True))
        prod_i = 0

        def pump(c_needed):
            nonlocal prod_i
            covered = -1
            while prod_i < len(sched) and covered < min(c_needed + LOOKAHEAD, C - 1):
                kind, v, counts = sched[prod_i]
                if kind == "bc":
                    emit_broadcast_pair(v)
                    covered = 2 * v + 1
                else:
                    emit_replicate(v)
                    if counts:
                        covered = v if v % 2 == 1 else v - 1
                prod_i += 1

        for p in range(N_PAIRS):
            pump(2 * p + 1)
            er2 = er_pair.pop(p)
            z2 = z_p.tile([D, 2 * N_LOC], BF16, tag="z2")
            if p < FRONT_PE:
                # startup: one multiply per expert so the first matmuls don't
                # wait for the whole pair's replication
                nc.vector.tensor_mul(z2[:, 0:N_LOC], xTb_s[:], er2[:, 0:N_LOC])
                nc.vector.tensor_mul(z2[:, N_LOC:], xTb_s[:], er2[:, N_LOC:])
            elif _pair_is_gp(p):
                nc.gpsimd.tensor_mul(z2[:], x_rep2, er2[:])
            else:
                nc.vector.tensor_mul(z2[:], x_rep2, er2[:])
            for half in range(2):
                c = 2 * p + half
                wv_c = wv_s[:, c * P:(c + 1) * P]
                for j in range(NCH):
                    js = slice(half * N_LOC + j * CHUNK,
                               half * N_LOC + (j + 1) * CHUNK)
                    ps = slice(j * CHUNK, (j + 1) * CHUNK)
                    nc.tensor.matmul(
                        po[:, ps], wv_c, z2[:, js],
                        start=(c == 0), stop=False, skip_group_check=True,
                    )

        # ---- Ov term closes each chunk's accumulation group; normalize and
        # store per chunk so the tail pipelines across PE/DVE/DMA ----
        out_s = out_p.tile([P, N_LOC], F32, tag="out")
        for j in range(NCH):
            js = slice(j * CHUNK, (j + 1) * CHUNK)
            nc.tensor.matmul(
                po[:, js], ov_s[:], eT_s[:, js],
                start=False, stop=True, skip_group_check=True,
            )
            nc.vector.tensor_mul(out_s[:, js], po[:, js], rzrep_s[:, js])
            nc.sync.dma_start(outT[:, js], out_s[:, js])


def build_nc():
    nc = bass.Bass(target_bir_lowering=False, trn_type="TRN2")
    aps = {
        "xT": nc.dram_tensor("xT", [D, N_LOC], F32R, kind="ExternalInput").ap(),
        "xTb": nc.dram_tensor("xTb", [D, N_LOC], BF16, kind="ExternalInput").ap(),
        "wvT": nc.dram_tensor("wvT", [D, C * P], BF16, kind="ExternalInput").ap(),
        "selT": nc.dram_tensor("selT", [C, C], BF16, kind="ExternalInput").ap(),
        "c2sT": nc.dram_tensor("c2sT", [D, C], F32R, kind="ExternalInput").ap(),
        "nccs": nc.dram_tensor("nccs", [C, 1], F32, kind="ExternalInput").ap(),
        "ov": nc.dram_tensor("ov", [C, P], BF16, kind="ExternalInput").ap(),
        "outT": nc.dram_tensor("outT", [P, N_LOC], F32, kind="ExternalOutput").ap(),
    }
    with tile.TileContext(nc) as tc:
        _emit_kernel(tc, aps)
    _dedup_ldweights(nc)
    _legalize_waits(nc)
    return nc


_CACHE = {}


def _get_nc():
    key = (PE_PAIRS, LOOKAHEAD)
    if key not in _CACHE:
        _CACHE[key] = build_nc()
    return _CACHE[key]


def kernel(x, ctrs, Wv, Ov, s, _spmd_kwargs=None):
    import ml_dtypes
    from concourse.bass_utils import run_bass_kernel_spmd

    bf16 = ml_dtypes.bfloat16
    x = np.ascontiguousarray(x, dtype=np.float32)
    ctrs = np.asarray(ctrs, dtype=np.float32)
    Wv = np.asarray(Wv, dtype=np.float32)
    Ov = np.ascontiguousarray(Ov, dtype=np.float32)
    s = np.asarray(s, dtype=np.float32)

    # host-side prep of the small routing constants
    c2sT = np.ascontiguousarray((2.0 * ctrs * s[None, :]).T)        # [g, c]
    nccs = np.ascontiguousarray(-((ctrs * ctrs) @ s)[:, None])      # [c, 1]
    wvT = np.ascontiguousarray(
        Wv.transpose(1, 0, 2).reshape(D, C * P).astype(bf16))       # [g, c*p]
    ov_b = np.ascontiguousarray(Ov.astype(bf16))
    selT = np.eye(C, dtype=np.float32).astype(bf16)

    in_maps = []
    for i in range(N_CORES):
        xi = x[i * N_LOC:(i + 1) * N_LOC]
        xiT = np.ascontiguousarray(xi.T)
        m = {
            "xT": xiT,
            "xTb": np.ascontiguousarray(xiT.astype(bf16)),
            "wvT": wvT,
            "selT": selT,
            "c2sT": c2sT,
            "nccs": nccs,
            "ov": ov_b,
        }
        in_maps.append(m)

    nc = _get_nc()
    for attempt in range(3):
        res = run_bass_kernel_spmd(
            nc, in_maps, core_ids=list(range(N_CORES)), **(_spmd_kwargs or {})
        )
        out = np.empty((N, P), dtype=np.float32)
        for i in range(N_CORES):
            out[i * N_LOC:(i + 1) * N_LOC] = res.results[i]["outT"].T
        kernel.last_result = res
        if np.isfinite(out).all():
            break
    return out
